# revision 4
# baseline (speedup 1.0000x reference)
"""Trainium2 Bass kernel for nn_MatchLoss.

Reference computation:
    an, bn, cn = l1_normalize(a|b|c, dim=C)        # per (b, h, w) column
    sim_ab = einsum('bchw,bcij->bhwij', an, bn)
    sim_ac = einsum('bchw,bcij->bhwij', an, cn)
    out = mean(|sim_ac - sim_ab|)                   # scalar

Algebraic restructure used here (per batch, flattening hw -> 4096):
    sim_ac - sim_ab = an^T @ (cn - bn) = diag(1/na) @ (a^T @ D),
        D = c * diag(1/nc) - b * diag(1/nb)
    loss_part = sum_q (1/na[q]) * sum_p |(a^T D)[q, p]|
so `a` is never normalized on-device; its norm is applied to the
per-query rowsums after the abs-reduce.

Sharding: 8 cores = 2 batches x 4 slices of the p (=ij) axis.  Each core
gets the full `a` for its batch (128 x 4096) plus a 1024-column slice of
b and c, computes rowsums of |a^T D_slice| scaled by 1/na into a
(128 x 32) partial, and the host sums the 8 partials.

Matmul inputs are bf16 (PE at 1 cycle/row); all accumulation (PSUM,
rowsums, norms) is fp32.  The only bf16 roundings are of |x| before the
norm sums, of 1/nb, 1/nc before the column scaling, and of a and D
before the big matmul - each perturbs the 33.5M-term mean by ~1e-4
relative, far inside fp32-envelope tolerances.
"""

import numpy as np

try:
    import concourse.bacc as bacc
    import concourse.tile as tile
    import concourse.mybir as mybir
    from concourse import bass_utils
except ImportError:  # pragma: no cover - fallback for bare containers
    import sys

    sys.path.insert(0, "/opt/trn_rl_repo")
    import concourse.bacc as bacc
    import concourse.tile as tile
    import concourse.mybir as mybir
    from concourse import bass_utils

B, C, H, W = 2, 128, 64, 64
HW = H * W              # 4096 (q axis, and full p axis)
N_CORES = 8
PSL = HW // 4           # 1024: per-core p-slice
QT = 128                # q tile (partition dim of PSUM result)
NQT = HW // QT          # 32 q tiles
CH = 512                # matmul moving chunk (one PSUM bank of fp32)

_F32 = mybir.dt.float32
_BF16 = mybir.dt.bfloat16
_AX = mybir.AxisListType
_AF = mybir.ActivationFunctionType
_OP = mybir.AluOpType


def _emit(tc, a_d, b_d, c_d, o_d):
    nc = tc.nc

    import contextlib

    with contextlib.ExitStack() as ctx:
        ctx.enter_context(
            nc.allow_low_precision(
                reason="bf16 matmul inputs; all accumulation stays fp32"
            )
        )
        sb = ctx.enter_context(tc.tile_pool(name="sb", bufs=1))

        A = sb.tile([C, HW], _BF16)
        absA = sb.tile([C, HW], _BF16)
        Bs = sb.tile([C, PSL], _F32)
        Cs = sb.tile([C, PSL], _F32)
        absB = sb.tile([C, PSL], _BF16)
        absC = sb.tile([C, PSL], _BF16)
        D = sb.tile([C, PSL], _BF16)
        t1 = sb.tile([C, PSL], _F32)
        t2 = sb.tile([C, PSL], _F32)
        ones_col = sb.tile([C, 1], _BF16)
        ones_row = sb.tile([1, C], _BF16)
        zeros_col = sb.tile([C, 1], _F32)
        rnb = sb.tile([1, PSL], _BF16)
        rnc = sb.tile([1, PSL], _BF16)
        rna = sb.tile([C, NQT], _F32)
        rs = sb.tile([C, NQT], _F32)
        res = sb.tile([C, NQT], _F32)
        trash = sb.tile([C, PSL], _F32)

        # --- input DMAs (b/c first: they gate the critical path to D) ---
        nc.sync.dma_start(Bs[:], b_d)
        nc.sync.dma_start(Cs[:], c_d)
        for i in range(4):
            sl = slice(i * PSL, (i + 1) * PSL)
            nc.gpsimd.dma_start(A[:, sl], a_d[:, sl])  # casts f32 -> bf16

        nc.vector.memset(ones_col[:], 1.0)
        nc.vector.memset(ones_row[:], 1.0)
        nc.vector.memset(zeros_col[:], 0.0)

        # --- |b|, |c|, |a| (bf16 outputs feeding the norm matmuls) ---
        nc.scalar.activation(absB[:], Bs[:], _AF.Abs, bias=zeros_col[:])
        nc.scalar.activation(absC[:], Cs[:], _AF.Abs, bias=zeros_col[:])
        for i in range(4):
            sl = slice(i * PSL, (i + 1) * PSL)
            nc.scalar.activation(absA[:, sl], A[:, sl], _AF.Abs, bias=zeros_col[:])

        with (
            tc.tile_pool(name="rows_ps", bufs=1, space="PSUM") as rows_ps,
            tc.tile_pool(name="bc_ps", bufs=2, space="PSUM") as bc_ps,
            tc.tile_pool(name="na_ps", bufs=1, space="PSUM") as na_ps,
        ):
            # column L1 norms of b,c: ones^T @ |x| -> (1, PSL) rows
            nbrow = rows_ps.tile([1, PSL], _F32)
            ncrow = rows_ps.tile([1, PSL], _F32)
            for j in range(PSL // CH):
                sl = slice(j * CH, (j + 1) * CH)
                nc.tensor.matmul(
                    nbrow[0:1, sl],
                    lhsT=ones_col[:],
                    rhs=absB[:, sl],
                    start=True,
                    stop=True,
                )
                nc.tensor.matmul(
                    ncrow[0:1, sl],
                    lhsT=ones_col[:],
                    rhs=absC[:, sl],
                    start=True,
                    stop=True,
                )
            nc.vector.reciprocal(rnb[:], nbrow[:])
            nc.vector.reciprocal(rnc[:], ncrow[:])

            # na: per-q-tile column norms of a, landing with q on partitions
            na = na_ps.tile([C, NQT], _F32)
            for t in range(NQT):
                nc.tensor.matmul(
                    na[:, t : t + 1],
                    lhsT=absA[:, t * QT : (t + 1) * QT],
                    rhs=ones_col[:],
                    start=True,
                    stop=True,
                )
            nc.vector.reciprocal(rna[:], na[:])

            # broadcast 1/nb, 1/nc across partitions (K=1 outer product),
            # then D = b * rb - c * rc
            for j in range(PSL // CH):
                sl = slice(j * CH, (j + 1) * CH)
                rb_bc = bc_ps.tile([C, CH], _F32, tag="bc")
                nc.tensor.matmul(
                    rb_bc[:],
                    lhsT=ones_row[:],
                    rhs=rnb[0:1, sl],
                    start=True,
                    stop=True,
                )
                rc_bc = bc_ps.tile([C, CH], _F32, tag="bc")
                nc.tensor.matmul(
                    rc_bc[:],
                    lhsT=ones_row[:],
                    rhs=rnc[0:1, sl],
                    start=True,
                    stop=True,
                )
                nc.vector.tensor_mul(t1[:, sl], Bs[:, sl], rb_bc[:])
                nc.vector.tensor_mul(t2[:, sl], Cs[:, sl], rc_bc[:])
                nc.vector.tensor_sub(D[:, sl], t1[:, sl], t2[:, sl])

        # --- main loop: M = a_tile^T @ D, rowsum(|M|) ---
        with tc.tile_pool(name="m_ps", bufs=2, space="PSUM") as m_ps:
            for st in range(NQT // 2):  # super-tile: 2 q-tiles x PSL
                M = m_ps.tile([C, 2, PSL], _F32)
                for qi in range(2):
                    t = st * 2 + qi
                    for j in range(PSL // CH):
                        nc.tensor.matmul(
                            M[:, qi, j * CH : (j + 1) * CH],
                            lhsT=A[:, t * QT : (t + 1) * QT],
                            rhs=D[:, j * CH : (j + 1) * CH],
                            start=True,
                            stop=True,
                        )
                if st % 2 == 0:
                    nc.vector.tensor_reduce(
                        out=rs[:, st * 2 : st * 2 + 2],
                        in_=M[:],
                        axis=_AX.X,
                        op=_OP.add,
                        apply_absolute_value=True,
                    )
                else:
                    for qi in range(2):
                        t = st * 2 + qi
                        nc.scalar.activation(
                            trash[:],
                            M[:, qi, :],
                            _AF.Abs,
                            bias=zeros_col[:],
                            accum_out=rs[:, t : t + 1],
                        )

        # --- scale rowsums by 1/na and write out ---
        nc.vector.tensor_mul(res[:], rs[:], rna[:])
        nc.sync.dma_start(o_d, res[:])


def _build():
    nc = bacc.Bacc(
        "TRN2", target_bir_lowering=False, debug=False, num_devices=N_CORES
    )
    a_d = nc.dram_tensor("a_full", (C, HW), _F32, kind="ExternalInput").ap()
    b_d = nc.dram_tensor("b_sl", (C, PSL), _F32, kind="ExternalInput").ap()
    c_d = nc.dram_tensor("c_sl", (C, PSL), _F32, kind="ExternalInput").ap()
    o_d = nc.dram_tensor("out", (C, NQT), _F32, kind="ExternalOutput").ap()
    with tile.TileContext(nc) as tc:
        _emit(tc, a_d, b_d, c_d, o_d)
    nc.finalize()
    return nc


_NC_CACHE = {}


def _get_nc():
    if "nc" not in _NC_CACHE:
        _NC_CACHE["nc"] = _build()
    return _NC_CACHE["nc"]


def _in_maps(a, b, c):
    a = np.ascontiguousarray(np.asarray(a, dtype=np.float32).reshape(B, C, HW))
    b = np.ascontiguousarray(np.asarray(b, dtype=np.float32).reshape(B, C, HW))
    c = np.ascontiguousarray(np.asarray(c, dtype=np.float32).reshape(B, C, HW))
    maps = []
    for core in range(N_CORES):
        bi, pi = divmod(core, 4)
        sl = slice(pi * PSL, (pi + 1) * PSL)
        maps.append(
            {
                "a_full": a[bi],
                "b_sl": np.ascontiguousarray(b[bi, :, sl]),
                "c_sl": np.ascontiguousarray(c[bi, :, sl]),
            }
        )
    return maps


def kernel(a, b, c):
    nc = _get_nc()
    res = bass_utils.run_bass_kernel_spmd(
        nc, _in_maps(a, b, c), core_ids=list(range(N_CORES))
    )
    total = np.float64(0.0)
    for core in range(N_CORES):
        total += np.sum(res.results[core]["out"], dtype=np.float64)
    return np.float32(total / (B * HW * HW))
